# revision 32
# baseline (speedup 1.0000x reference)
"""Trainium2 Bass kernel for nn_DeepONetCfCDecoder.

Strategy (8 NeuronCores, data-parallel over queries, time-banded):
  * Host: searchsorted -> per-query time-bucket idx; stable-sort queries by
    idx; split into 8 equal rank-chunks (one per core).  Each core gets a
    contiguous band of h_states buckets plus its queries packed into tiles of
    128 that each cover a window of <= G consecutive buckets.
  * Device: per core, build K^T / V tables for its band with two matmuls
    (weights pre-folded on host: W_k = btok_w@bk_w, W_v = btok_w@bv_w; all
    additive K/V biases either cancel in softmax or fold to a constant cv),
    then per tile: trunk MLP (fourier + time + component embedding), q
    projection, block-masked attention against a dynamically-offset window of
    the K/V tables (matmul rhs supports register offsets; no slab copy),
    context MLP, and the rank-basis contraction.
  * rel_bias of the reference is structurally zero (LayerNorm over a
    singleton axis -> 0; rb1 = rb2 = 0), and constant-per-row score offsets
    cancel in softmax, so the whole relative-position branch is dropped.
  * Scalar engine only uses {Exp, Tanh, Identity, Copy} (all in one act
    table -> no ACT_TABLE_LOADs): sin via a DVE polynomial on the wrapped
    phase, silu via 0.5*x*(1+tanh(x/2)).  The window slab copy is split
    across DVE and Act (one dynamic-offset AP per engine per tile).
"""

import sys

sys.path.insert(0, "/opt/trn_rl_repo")

import numpy as np
import ml_dtypes

import concourse.bass as bass
import concourse.mybir as mybir
import concourse.tile as tile
import bass_rust as _bass_rust
from concourse.bass_utils import run_bass_kernel_spmd

BF16 = ml_dtypes.bfloat16
F32 = mybir.dt.float32
BF = mybir.dt.bfloat16
AF = mybir.ActivationFunctionType
ALU = mybir.AluOpType

N, K, T, D = 8192, 64, 512, 256
H, RANK, DTDIM, FH, L = 256, 256, 32, 8, 1.0
NCORES = 8
G = 12          # bucket slots per tile window (must be even)
P = 128         # queries per tile
NEG = -30000.0  # additive mask value

# sin(2*pi*u) ~ u*(S0 + S1 u^2 + S2 u^4 + S3 u^6) on [-0.5, 0.5]
S0, S1, S2, S3 = 6.27973012, -41.13623479, 78.32684839, -57.1154045

# consts row layout (f32)
C_HARM0, C_HARM1, C_IOTA3, C_CS, C_CB = 0, 8, 16, 19, 22
C_TPW, C_TPB, C_EMB0, C_EMB1, C_EMB2 = 25, 57, 89, 97, 105
C_OFFS = 113
CW = 145


def _pack(t_q, sensor_time):
    """Sort queries by bucket, chunk to cores, pack 128-query tiles."""
    idx = np.clip(np.searchsorted(sensor_time, t_q, side="right") - 1, 0, T - 1)
    order = np.argsort(idx, kind="stable")
    per_core = N // NCORES
    raw = []
    maxB = maxTPC = 0
    for i in range(NCORES):
        sel = order[i * per_core:(i + 1) * per_core]
        bidx = idx[sel]
        lo = int(bidx[0])
        Bc = int(bidx[-1]) - lo + 1
        tiles = []
        pos = 0
        while pos < len(sel):
            b0 = int(bidx[pos]) - lo
            s = b0 - (b0 % 2)
            take, g = [], []
            while pos < len(sel) and len(take) < P and int(bidx[pos]) - lo < s + G:
                take.append(sel[pos])
                g.append(int(bidx[pos]) - lo - s)
                pos += 1
            nreal = len(take)
            while len(take) < P:
                take.append(take[-1])
                g.append(g[-1])
            tiles.append([s, np.array(take), np.array(g, np.int64), nreal])
        raw.append((lo, Bc, tiles))
        maxB = max(maxB, Bc)
        maxTPC = max(maxTPC, len(tiles))
    B = max(maxB, G)
    B = (B + 7) // 8 * 8          # even + 512-divisible free chunks
    TPC = maxTPC
    cores = []
    for lo, Bc, tiles in raw:
        fixed = []
        for s, q, g, nr in tiles:
            s2 = min(s, B - G)
            fixed.append((s2, q, g + (s - s2), nr))
        while len(fixed) < TPC:
            fixed.append((0, fixed[-1][1], np.zeros(P, np.int64), 0))
        cores.append((lo, fixed))
    return cores, B, TPC, idx


def _build(B, TPC):
    B64 = B * 64
    nc = bass.Bass()

    def inp(name, shape, dt=BF):
        return nc.declare_dram_parameter(name, list(shape), dt, isOutput=False)

    ht_d = inp("ht", [128, 2 * B64])
    wk_d = inp("wk", [128, 512])
    wv_d = inp("wv", [128, 512])
    trunkw_d = inp("trunkw", [72, 256])
    bq_d = inp("bqw", [128, 512])
    cw1_d = inp("cw1w", [128, 512])
    cw2_d = inp("cw2w", [128, 512])
    tow_d = inp("tow", [128, 1536])
    bpw_d = inp("bpw", [128, 1536])
    rowb_d = inp("rowb", [1, 1536])
    ones_d = inp("ones", [1, 128])
    expander_d = inp("expander", [12, 768])
    ppb_d = inp("ppb", [128, 12], F32)
    ident_d = inp("ident", [128, 128])
    onesf_d = inp("onesf", [1, 128], F32)
    cvrow_d = inp("cvrow", [1, 256], F32)
    consts_d = inp("consts", [1, CW], F32)
    iota12_d = inp("iota12", [12, 1], F32)
    stw_d = inp("stw", [1, B], F32)
    qmeta_d = inp("qmeta", [TPC, 128, 4], F32)
    grow_d = inp("grow", [TPC, 128], F32)
    moff_d = inp("moff", [1, TPC * 2], mybir.dt.int32)
    out_d = nc.declare_dram_parameter("out", [128, TPC], F32, isOutput=True)

    with tile.TileContext(nc) as tc:
        with (
            tc.tile_pool(name="const", bufs=1) as cp,
            tc.tile_pool(name="work", bufs=4) as wp,
            tc.tile_pool(name="work3", bufs=4) as wp3,
            tc.tile_pool(name="psum", bufs=2, space="PSUM") as pp,
        ):
            _ptc = [0]
            def ptile(shape, dt, tag, **kw):
                if "name" not in kw:
                    _ptc[0] += 1
                    kw["name"] = f"{tag}_{_ptc[0]}"
                return pp.tile(shape, dt, tag=tag, **kw)
            def act_silu(out_ap, in_ap, fullb_ap, halfb_ap, tag):
                # silu(x+b) = 0.5(x+b) * (1 + tanh(0.5(x+b)))
                t = wp.tile([128, 128], F32, tag=tag + "_t")
                nc.scalar.activation(t[:], in_ap, AF.Tanh, bias=halfb_ap, scale=0.5)
                v = wp.tile([128, 128], F32, tag=tag + "_v")
                nc.vector.tensor_scalar(v[:], in_ap, fullb_ap, 0.5, ALU.add, ALU.mult)
                nc.vector.scalar_tensor_tensor(
                    out_ap, t[:], 1.0, v[:], ALU.add, ALU.mult)

            # ---------------- startup: constants & weights ----------------
            id_bf = cp.tile([128, 128], BF, tag="id_bf")
            nc.sync.dma_start(id_bf[:], ident_d[:])
            onesf = cp.tile([1, 128], F32, tag="onesf")
            nc.sync.dma_start(onesf[:], onesf_d[:])

            ht_sb = cp.tile([128, 2 * B64], BF, tag="ht")
            nc.sync.dma_start(ht_sb[:], ht_d[:])
            wk_sb = cp.tile([128, 512], BF, tag="wk")
            nc.sync.dma_start(wk_sb[:], wk_d[:])
            wv_sb = cp.tile([128, 512], BF, tag="wv")
            nc.sync.dma_start(wv_sb[:], wv_d[:])
            trunkw_sb = cp.tile([72, 256], BF, tag="trunkw")
            nc.sync.dma_start(trunkw_sb[:], trunkw_d[:])
            bq_sb = cp.tile([128, 512], BF, tag="bq")
            nc.sync.dma_start(bq_sb[:], bq_d[:])
            cw1_sb = cp.tile([128, 512], BF, tag="cw1")
            nc.sync.dma_start(cw1_sb[:], cw1_d[:])
            cw2_sb = cp.tile([128, 512], BF, tag="cw2")
            nc.sync.dma_start(cw2_sb[:], cw2_d[:])
            tow_sb = cp.tile([128, 1536], BF, tag="tow")
            nc.sync.dma_start(tow_sb[:], tow_d[:])
            bpw_sb = cp.tile([128, 1536], BF, tag="bpw")
            nc.sync.dma_start(bpw_sb[:], bpw_d[:])
            rowb_sb = cp.tile([1, 1536], BF, tag="rowb")
            nc.sync.dma_start(rowb_sb[:], rowb_d[:])
            ones1 = cp.tile([1, 128], BF, tag="ones1")
            nc.sync.dma_start(ones1[:], ones_d[:])
            expander_sb = cp.tile([12, 768], BF, tag="expander")
            nc.sync.dma_start(expander_sb[:], expander_d[:])
            ppb_sb = cp.tile([128, 12], F32, tag="ppb")
            nc.sync.dma_start(ppb_sb[:], ppb_d[:])
            iota12_sb = cp.tile([12, 1], F32, tag="iota12")
            nc.sync.dma_start(iota12_sb[:], iota12_d[:])
            moff_sb = cp.tile([1, TPC * 2], mybir.dt.int32, tag="moff")
            nc.sync.dma_start(moff_sb[:], moff_d[:])

            def pe_bcast(row_ap, width, dst_tile):
                psb = ptile([128, 512], F32, tag="scps")
                for w0 in range(0, width, 512):
                    w = min(512, width - w0)
                    nc.tensor.matmul(psb[:, 0:w], onesf[:], row_ap[0:1, w0:w0 + w],
                                     start=True, stop=True)
                    nc.vector.tensor_copy(dst_tile[:, w0:w0 + w], psb[:, 0:w])

            cvrow_sb = cp.tile([1, 256], F32, tag="cvrow")
            nc.sync.dma_start(cvrow_sb[:], cvrow_d[:])
            cv_rep = cp.tile([128, 256], F32, tag="cv_rep")
            pe_bcast(cvrow_sb[:], 256, cv_rep)
            crow_sb = cp.tile([1, CW], F32, tag="crow")
            nc.sync.dma_start(crow_sb[:], consts_d[:])
            crep = cp.tile([128, CW], F32, tag="crep")
            pe_bcast(crow_sb[:], CW, crep)
            stwrow_sb = cp.tile([1, B], F32, tag="stwrow")
            nc.sync.dma_start(stwrow_sb[:], stw_d[:])
            stw_rep = cp.tile([128, B], F32, tag="stw_rep")
            pe_bcast(stwrow_sb[:], B, stw_rep)
            stm0_rep = cp.tile([128, B], F32, tag="stm0_rep")
            nc.vector.tensor_scalar(
                stm0_rep[:], stw_rep[:], stw_rep[:, 0:1], None, ALU.subtract
            )
            out_acc = cp.tile([128, TPC], F32, tag="out_acc")

            # ---------------- phase 1: K^T and V tables ----------------
            # combined interleaved table: per 2-bucket unit u (=128 j-rows):
            #   cols [512u,512u+128) = K^T chunk0, +128..256 = K^T chunk1,
            #   +256..512 = V rows of unit u.  One dynamic window covers a tile.
            ctab = cp.tile([128, (B // 2) * 512], BF, tag="ctab")
            ctab_v = ctab[:].rearrange("p (u blk) -> p u blk", blk=512)
            ei = 0
            for ch in range(2):
                for f0 in range(0, B64, 512):
                    ps = ptile([128, 512], F32, tag="scps")
                    for dch in range(2):
                        nc.tensor.matmul(
                            ps[:, 0:512],
                            wk_sb[:, (dch * 2 + ch) * 128:(dch * 2 + ch + 1) * 128],
                            ht_sb[:, dch * B64 + f0:dch * B64 + f0 + 512],
                            start=(dch == 0),
                            stop=(dch == 1),
                        )
                    dst = ctab_v[:, f0 // 128:f0 // 128 + 4, ch * 128:(ch + 1) * 128]
                    psv = ps[:, 0:512].rearrange("p (u blk) -> p u blk", blk=128)
                    if ei % 2 == 0:
                        nc.vector.tensor_copy(dst, psv)
                    else:
                        nc.scalar.activation(dst, psv, AF.Copy)
                    ei += 1
            for jp in range(B64 // 256):
                ps = ptile([128, 512], F32, tag="scps")
                for half in range(2):
                    js = 2 * jp + half
                    for dch in range(2):
                        nc.tensor.matmul(
                            ps[:, half * 256:(half + 1) * 256],
                            ht_sb[:, dch * B64 + js * 128:dch * B64 + (js + 1) * 128],
                            wv_sb[:, dch * 256:(dch + 1) * 256],
                            start=(dch == 0),
                            stop=(dch == 1),
                        )
                dst = ctab_v[:, 2 * jp:2 * jp + 2, 256:512]
                psv2 = ps[:].rearrange("p (u blk) -> p u blk", blk=256)
                if ei % 2 == 0:
                    nc.vector.tensor_copy(dst, psv2)
                else:
                    nc.scalar.activation(dst, psv2, AF.Copy)
                ei += 1

            # ---------------- phase 2: per-tile pipeline ----------------
            def layernorm(x, tag):
                mean = wp.tile([128, 1], F32, tag=tag + "_m")
                nc.vector.tensor_reduce(mean[:], x, mybir.AxisListType.X, ALU.add)
                nc.vector.tensor_scalar(mean[:], mean[:], 1.0 / 256, None, ALU.mult)
                cen = wp.tile([128, 256], F32, tag=tag + "_c")
                nc.vector.tensor_scalar(cen[:], x, mean[:], None, ALU.subtract)
                sq = wp.tile([128, 256], F32, tag=tag + "_s")
                vsum = wp.tile([128, 1], F32, tag=tag + "_v")
                nc.vector.scalar_tensor_tensor(
                    sq[:], cen[:], 1.0, cen[:], ALU.mult, ALU.mult, accum_out=vsum[:]
                )
                vv = wp.tile([128, 1], F32, tag=tag + "_vv")
                nc.vector.tensor_scalar(vv[:], vsum[:], 1.0 / 256, 1e-5, ALU.mult, ALU.add)
                t0 = wp.tile([128, 1], mybir.dt.int32, tag=tag + "_t0")
                nc.vector.tensor_scalar(t0[:], vv[:].bitcast(mybir.dt.int32), 1, None,
                                        ALU.arith_shift_right)
                y0i = wp.tile([128, 1], mybir.dt.int32, tag=tag + "_y0")
                nc.vector.tensor_scalar(y0i[:], t0[:], 0x5F3759DF, -1, ALU.subtract, ALU.mult)
                y0 = y0i[:].bitcast(F32)
                t1 = wp.tile([128, 1], F32, tag=tag + "_t1")
                nc.vector.tensor_tensor(t1[:], y0, y0, ALU.mult)
                t2 = wp.tile([128, 1], F32, tag=tag + "_t2")
                nc.vector.tensor_scalar(t2[:], t1[:], 0.5, None, ALU.mult)
                nc.vector.tensor_tensor(t2[:], t2[:], vv[:], ALU.mult)
                t3 = wp.tile([128, 1], F32, tag=tag + "_t3")
                nc.vector.tensor_scalar(t3[:], t2[:], 1.5, -1.0, ALU.subtract, ALU.mult)
                rstd = wp.tile([128, 1], F32, tag=tag + "_r")
                nc.vector.tensor_tensor(rstd[:], y0, t3[:], ALU.mult)
                ln = wp.tile([128, 256], BF, tag=tag + "_ln")
                nc.vector.tensor_scalar(ln[:], cen[:], rstd[:], None, ALU.mult)
                return ln

            def tile_body(m):
                qm = wp3.tile([128, 4], F32, tag="qm")
                nc.sync.dma_start(qm[:], qmeta_d[m])
                grow_sb = wp3.tile([1, 128], F32, tag="grow")
                nc.sync.dma_start(grow_sb[:], grow_d[m:m + 1, :])
                g12_ps = ptile([12, 128], F32, tag="early")
                nc.tensor.matmul(g12_ps[:], onesf[0:1, 0:12], grow_sb[:],
                                 start=True, stop=True)
                onehotT = wp.tile([12, 128], BF, tag="onehotT")
                nc.vector.tensor_scalar(
                    onehotT[:], g12_ps[:], iota12_sb[:], None, ALU.is_equal
                )

                # window slab: 2 dynamic-offset copies (DVE half + Act
                # half) -- one dynamic AP per engine per tile; registers for
                # dynamic APs are scarce (one pool per engine).
                offmax = ((B // 2) * 512 - 3072, (B // 2) * 512 - 2048)
                engs = ([mybir.EngineType.DVE], [mybir.EngineType.Activation])
                with tc.high_priority():
                    offs = [
                        nc.values_load(
                            moff_sb[0:1, 2 * m + k:2 * m + k + 1],
                            engines=engs[k],
                            min_val=0, max_val=offmax[k],
                            skip_runtime_bounds_check=True,
                        )
                        for k in range(2)
                    ]
                    cslab = wp.tile([128, 3072], BF, tag="cslab")
                    nc.vector.tensor_copy(cslab[:, 0:1024],
                                          ctab[:, bass.ds(offs[0], 1024)])
                    nc.scalar.activation(cslab[:, 1024:3072],
                                         ctab[:, bass.ds(offs[1], 2048)], AF.Copy)
                cslab_v = cslab[:].rearrange("p (u blk) -> p u blk", blk=512)
                yield

                # --- dt via masked max over the band's sensor times (gpsimd) ---
                tq = qm[:, 0:1]
                contrib = wp.tile([128, B], F32, tag="contrib")
                nc.vector.scalar_tensor_tensor(
                    contrib[:], stw_rep[:], tq, stm0_rep[:], ALU.is_le, ALU.mult)
                tmax = wp.tile([128, 1], F32, tag="tmax")
                nc.vector.tensor_reduce(tmax[:], contrib[:], mybir.AxisListType.X, ALU.max)
                dt = wp.tile([128, 1], F32, tag="dt")
                nc.vector.scalar_tensor_tensor(
                    dt[:], qm[:, 0:1], tmax[:], stw_rep[:, 0:1], ALU.subtract, ALU.subtract
                )
                nc.vector.tensor_scalar(dt[:], dt[:], 0.0, None, ALU.max)

                # --- oh [128,3] from component id (gpsimd) ---
                oh = wp.tile([128, 3], F32, tag="oh")
                nc.vector.tensor_scalar(
                    oh[:], crep[:, C_IOTA3:C_IOTA3 + 3], qm[:, 1:2], None, ALU.is_equal
                )

                # --- trunk features [128, 72] on gpsimd ---
                # u = [h*x, h*x+1/4, h*y, h*y+1/4]; wrap u to [-.5,.5); sin
                # via odd polynomial (scalar engine keeps one act table).
                feat = wp.tile([128, 72], BF, tag="feat")
                ang = wp.tile([128, 32], F32, tag="ang")
                nc.vector.tensor_scalar(
                    ang[:, 0:16], crep[:, C_HARM0:C_HARM0 + 16], qm[:, 2:3], None, ALU.mult
                )
                nc.vector.tensor_scalar(
                    ang[:, 16:32], crep[:, C_HARM0:C_HARM0 + 16], qm[:, 3:4], None, ALU.mult
                )
                nc.vector.tensor_tensor(ang[:], ang[:], crep[:, C_OFFS:C_OFFS + 32], ALU.add)
                MAGIC = float(1.5 * 2 ** 23)
                rnd = wp.tile([128, 32], F32, tag="rnd")
                nc.gpsimd.tensor_scalar(rnd[:], ang[:], MAGIC, MAGIC, ALU.add, ALU.subtract)
                nc.gpsimd.tensor_tensor(ang[:], ang[:], rnd[:], ALU.subtract)
                u2 = wp.tile([128, 32], F32, tag="u2")
                nc.gpsimd.tensor_tensor(u2[:], ang[:], ang[:], ALU.mult)
                sp = wp.tile([128, 32], F32, tag="sp")
                nc.gpsimd.tensor_scalar(sp[:], u2[:], S3, S2, ALU.mult, ALU.add)
                nc.gpsimd.tensor_tensor(sp[:], sp[:], u2[:], ALU.mult)
                nc.gpsimd.tensor_scalar(sp[:], sp[:], S1, None, ALU.add)
                nc.gpsimd.tensor_tensor(sp[:], sp[:], u2[:], ALU.mult)
                nc.gpsimd.tensor_scalar(sp[:], sp[:], S0, None, ALU.add)
                nc.gpsimd.tensor_tensor(feat[:, 0:32], sp[:], ang[:], ALU.mult)
                nc.vector.scalar_tensor_tensor(
                    feat[:, 32:64], crep[:, C_TPW:C_TPW + 32], dt[:],
                    crep[:, C_TPB:C_TPB + 32], ALU.mult, ALU.add)
                nc.vector.tensor_scalar(
                    feat[:, 64:72], crep[:, C_EMB0:C_EMB0 + 8], oh[:, 0:1], None, ALU.mult
                )
                nc.vector.scalar_tensor_tensor(
                    feat[:, 64:72], crep[:, C_EMB1:C_EMB1 + 8], oh[:, 1:2],
                    feat[:, 64:72], ALU.mult, ALU.add)
                nc.vector.scalar_tensor_tensor(
                    feat[:, 64:72], crep[:, C_EMB2:C_EMB2 + 8], oh[:, 2:3],
                    feat[:, 64:72], ALU.mult, ALU.add)
                yield

                # --- trunk MLP: featT -> trunkT -> silu ---
                tp1 = ptile([72, 128], BF, tag="tp")
                nc.tensor.transpose(tp1[:], feat[:], id_bf[:])
                featT = wp.tile([72, 128], BF, tag="featT")
                nc.scalar.activation(featT[:], tp1[:], AF.Copy)
                trunkT_ps = ptile([128, 256], F32, tag="early")
                for ich in range(2):
                    nc.tensor.matmul(
                        trunkT_ps[:, ich * 128:(ich + 1) * 128],
                        trunkw_sb[:, ich * 128:(ich + 1) * 128],
                        featT[:],
                        start=True, stop=True,
                    )
                featTs = wp.tile([128, 256], BF, tag="featTs")
                for ich in range(2):
                    act_silu(
                        featTs[:, ich * 128:(ich + 1) * 128],
                        trunkT_ps[:, ich * 128:(ich + 1) * 128],
                        ppb_sb[:, ich:ich + 1], ppb_sb[:, 6 + ich:7 + ich], "silu_t",
                    )
                yield

                # --- feat rows for LN (one merged transpose pair + copy) ---
                tpA = ptile([128, 768], BF, tag="tp")
                for ich in range(2):
                    nc.tensor.transpose(
                        tpA[:, ich * 128:(ich + 1) * 128],
                        featTs[:, ich * 128:(ich + 1) * 128], id_bf[:])
                feat_row = wp.tile([128, 256], BF, tag="feat_row")
                nc.vector.tensor_copy(feat_row[:], tpA[:, 0:256])

                lnf = layernorm(feat_row[:], "ln1")
                tpB = ptile([128, 768], BF, tag="tp")
                for ich in range(2):
                    nc.tensor.transpose(
                        tpB[:, ich * 128:(ich + 1) * 128],
                        lnf[:, ich * 128:(ich + 1) * 128], id_bf[:])
                lnT = wp.tile([128, 256], BF, tag="lnT")
                nc.scalar.activation(lnT[:], tpB[:, 0:256], AF.Copy)
                yield

                # --- q^T ---
                qT_ps = ptile([128, 256], F32, tag="early")
                for ich in range(2):
                    for hch in range(2):
                        nc.tensor.matmul(
                            qT_ps[:, ich * 128:(ich + 1) * 128],
                            bq_sb[:, (hch * 2 + ich) * 128:(hch * 2 + ich + 1) * 128],
                            lnT[:, hch * 128:(hch + 1) * 128],
                            start=(hch == 0), stop=(hch == 1),
                        )
                qT = wp.tile([128, 256], BF, tag="qT")
                for ich in range(2):
                    nc.scalar.activation(
                        qT[:, ich * 128:(ich + 1) * 128],
                        qT_ps[:, ich * 128:(ich + 1) * 128],
                        AF.Identity, bias=ppb_sb[:, 2 + ich:3 + ich], scale=1.0 / 16,
                    )
                yield

                # --- scores + additive block mask; K window via dynamic rhs ---
                expm = wp.tile([128, 768], BF, tag="expm")
                den2 = wp.tile([128, 2], F32, tag="den2")
                for i, (f0, fw, tg) in enumerate(
                        ((0, 512, "scps"), (512, 256, "late"))):
                    scp = ptile([128, fw], F32, tag=tg)
                    u0, u1 = f0 // 128, (f0 + fw) // 128
                    for dch in range(2):
                        nc.tensor.matmul(
                            scp[:],
                            qT[:, dch * 128:(dch + 1) * 128],
                            cslab_v[:, u0:u1, dch * 128:(dch + 1) * 128],
                            start=(dch == 0), stop=False,
                        )
                    nc.tensor.matmul(
                        scp[:],
                        onehotT[:],
                        expander_sb[:, f0:f0 + fw],
                        start=False, stop=True,
                    )
                    nc.scalar.activation(
                        expm[:, f0:f0 + fw], scp[:], AF.Exp,
                        accum_out=den2[:, i:i + 1],
                    )
                recip = wp.tile([128, 1], F32, tag="recip")
                nc.vector.tensor_tensor(recip[:], den2[:, 0:1], den2[:, 1:2], ALU.add)
                nc.vector.reciprocal(recip[:], recip[:])
                yield

                tpC = ptile([128, 768], BF, tag="tp")
                for j in range(6):
                    nc.tensor.transpose(
                        tpC[:, j * 128:(j + 1) * 128],
                        expm[:, j * 128:(j + 1) * 128], id_bf[:])
                expT = wp.tile([128, 768], BF, tag="expT")
                nc.vector.tensor_copy(expT[:], tpC[:])
                yield

                ctx_ps = ptile([128, 256], F32, tag="late")
                for j in range(6):
                    nc.tensor.matmul(
                        ctx_ps[:],
                        expT[:, j * 128:(j + 1) * 128],
                        cslab_v[:, j, 256:512],
                        start=(j == 0), stop=(j == 5),
                    )
                ctx = wp.tile([128, 256], F32, tag="ctx")
                nc.vector.scalar_tensor_tensor(
                    ctx[:], ctx_ps[:], recip[:], cv_rep[:], ALU.mult, ALU.add
                )
                yield

                # --- trunk basis tb (to_w); bias added with the psum->sbuf op ---
                tb_sb = wp.tile([128, 768], BF, tag="tb_sb")
                for f0, fw, tg in ((0, 512, "scps"), (512, 256, "early")):
                    tbp = ptile([128, fw], F32, tag=tg)
                    for hch in range(2):
                        nc.tensor.matmul(
                            tbp[:],
                            featTs[:, hch * 128:(hch + 1) * 128],
                            tow_sb[:, hch * 768 + f0:hch * 768 + f0 + fw],
                            start=(hch == 0), stop=False,
                        )
                    nc.tensor.matmul(
                        tbp[:], ones1[:],
                        rowb_sb[0:1, 768 + f0:768 + f0 + fw],
                        start=False, stop=True,
                    )
                    nc.scalar.activation(tb_sb[:, f0:f0 + fw], tbp[:], AF.Copy)
                yield

                # --- context MLP ---
                lnc = layernorm(ctx[:], "ln2")
                tpD = ptile([128, 768], BF, tag="tp")
                for ich in range(2):
                    nc.tensor.transpose(
                        tpD[:, ich * 128:(ich + 1) * 128],
                        lnc[:, ich * 128:(ich + 1) * 128], id_bf[:])
                lncT = wp.tile([128, 256], BF, tag="lncT")
                nc.vector.tensor_copy(lncT[:], tpD[:, 0:256])
                h1_ps = ptile([128, 256], F32, tag="late")
                for ich in range(2):
                    for hch in range(2):
                        nc.tensor.matmul(
                            h1_ps[:, ich * 128:(ich + 1) * 128],
                            cw1_sb[:, (hch * 2 + ich) * 128:(hch * 2 + ich + 1) * 128],
                            lncT[:, hch * 128:(hch + 1) * 128],
                            start=(hch == 0), stop=(hch == 1),
                        )
                h1T = wp.tile([128, 256], BF, tag="h1T")
                for ich in range(2):
                    act_silu(
                        h1T[:, ich * 128:(ich + 1) * 128],
                        h1_ps[:, ich * 128:(ich + 1) * 128],
                        ppb_sb[:, 4 + ich:5 + ich], ppb_sb[:, 8 + ich:9 + ich], "silu_h",
                    )
                yield
                mlp_ps = ptile([128, 256], F32, tag="late")
                for ich in range(2):
                    nc.tensor.matmul(
                        mlp_ps[:],
                        h1T[:, ich * 128:(ich + 1) * 128],
                        cw2_sb[:, ich * 256:(ich + 1) * 256],
                        start=(ich == 0), stop=(ich == 1),
                    )
                # cb2 is folded into bp_b_eff on the host; ctx3 = ctx + mlp
                ctx3 = wp.tile([128, 256], BF, tag="ctx3")
                nc.vector.tensor_tensor(ctx3[:], mlp_ps[:], ctx[:], ALU.add)
                tpE = ptile([128, 768], BF, tag="tp")
                for ich in range(2):
                    nc.tensor.transpose(
                        tpE[:, ich * 128:(ich + 1) * 128],
                        ctx3[:, ich * 128:(ich + 1) * 128], id_bf[:])
                ctx3T = wp.tile([128, 256], BF, tag="ctx3T")
                nc.scalar.activation(ctx3T[:], tpE[:, 0:256], AF.Copy)
                yield

                # --- branch basis + rank contraction per component ---
                s3 = wp.tile([128, 3], F32, tag="s3")
                scratch = wp.tile([128, 256], F32, tag="scratch")
                bps_l = []
                for _c in range(3):
                    bps_l.append(ptile([128, 256], F32, tag="late", name=f"bps{_c}_{m}"))
                for hch in range(2):
                    for comp in range(3):
                        nc.tensor.matmul(
                            bps_l[comp][:],
                            ctx3T[:, hch * 128:(hch + 1) * 128],
                            bpw_sb[:, hch * 768 + comp * 256:hch * 768 + (comp + 1) * 256],
                            start=(hch == 0), stop=False,
                        )
                for comp in range(3):
                    nc.tensor.matmul(
                        bps_l[comp][:], ones1[:],
                        rowb_sb[0:1, comp * 256:(comp + 1) * 256],
                        start=False, stop=True,
                    )
                    nc.vector.scalar_tensor_tensor(
                        scratch[:], bps_l[comp][:], 1.0,
                        tb_sb[:, comp * 256:(comp + 1) * 256],
                        ALU.mult, ALU.mult, accum_out=s3[:, comp:comp + 1],
                    )

                # out = sum_i oh_i * (s3_i * cs_i + cb_i)
                w3 = wp.tile([128, 3], F32, tag="w3")
                nc.vector.tensor_tensor(w3[:], s3[:], crep[:, C_CS:C_CS + 3], ALU.mult)
                nc.vector.tensor_tensor(w3[:], w3[:], crep[:, C_CB:C_CB + 3], ALU.add)
                scr3 = wp.tile([128, 3], F32, tag="scr3")
                nc.vector.scalar_tensor_tensor(
                    scr3[:], w3[:], 1.0, oh[:], ALU.mult, ALU.mult,
                    accum_out=out_acc[:, m:m + 1],
                )

            # drive tile bodies round-robin so each engine's in-order queue
            # interleaves independent tiles (avoids head-of-line blocking)
            from collections import deque
            live = deque()
            nxt = 0
            while live or nxt < TPC:
                while len(live) < 3 and nxt < TPC:
                    live.append(tile_body(nxt))
                    nxt += 1
                g = live.popleft()
                try:
                    next(g)
                    live.append(g)
                except StopIteration:
                    pass
            nc.sync.dma_start(out_d[:], out_acc[:])
    # split multi-waits: HW allows at most one sync wait per instruction
    _bass_rust.move_matmul_waits_to_ldweights(nc.m)
    _bass_rust.generate_event_semaphores(nc)
    return nc


def _prepare(inputs):
    ins = {k: np.asarray(v) for k, v in inputs.items()}
    t_q = ins["t_q"].astype(np.float32)
    st = ins["sensor_time"].astype(np.float32)
    xy = ins["xy"].astype(np.float32)
    c = ins["c"].astype(np.float32)
    h = ins["h_states"].astype(np.float32)

    cores, B, TPC, idx = _pack(t_q, st)
    B64 = B * 64

    # ---- host-side parameter folds ----
    W_k = ins["btok_w"] @ ins["bk_w"]
    W_v = ins["btok_w"] @ ins["bv_w"]
    cv = ins["btok_b"] @ ins["bv_w"] + ins["bv_b"]
    bq_w_eff = ins["bn_g"][:, None] * ins["bq_w"]
    bq_b_eff = ins["bn_b"] @ ins["bq_w"] + ins["bq_b"]
    cw1_eff = ins["cln_g"][:, None] * ins["cw1"]
    cb1_eff = ins["cln_b"] @ ins["cw1"] + ins["cb1"]
    bp_b_eff = ins["cb2"] @ ins["bp_w"] + ins["bp_b"]
    temp = float(np.exp(ins["log_temp"][0]))

    def chunk2(w):  # [256, X] -> [128, 2*X]  (col = dch*X + x)
        x = w.shape[1]
        return np.ascontiguousarray(
            w.reshape(2, 128, x).transpose(1, 0, 2).reshape(128, 2 * x)
        ).astype(BF16)

    def chunk22(w):  # [256, 256] -> [128, 512]  (col = (dch*2+ich)*128 + i)
        return np.ascontiguousarray(
            w.reshape(2, 128, 2, 128).transpose(1, 0, 2, 3).reshape(128, 512)
        ).astype(BF16)

    wk_h = chunk22(W_k)
    bq_h = chunk22(bq_w_eff)
    cw1_h = chunk22(cw1_eff)
    wv_h = chunk2(W_v)
    cw2_h = chunk2(ins["cw2"])
    tow_h = chunk2(ins["to_w"])
    bpw_h = chunk2(ins["bp_w"])
    trunkw_h = ins["trunk_in_w"].astype(BF16)
    rowb_h = np.concatenate([bp_b_eff, ins["to_b"]]).astype(np.float32)[None, :]
    tib = ins["trunk_in_b"].astype(np.float32)
    cb1f = cb1_eff.astype(np.float32)
    ppb_h = np.ascontiguousarray(np.stack([
        tib[0:128], tib[128:256],
        bq_b_eff[0:128] / 16.0, bq_b_eff[128:256] / 16.0,
        cb1f[0:128], cb1f[128:256],
        0.5 * tib[0:128], 0.5 * tib[128:256],
        0.5 * cb1f[0:128], 0.5 * cb1f[128:256],
        np.zeros(128, np.float32), np.zeros(128, np.float32),
    ]).T).astype(np.float32)
    cvrow_h = cv.astype(np.float32)[None, :]
    harm = np.arange(1, FH + 1, dtype=np.float32)
    consts_h = np.zeros((1, CW), np.float32)
    consts_h[0, C_HARM0:C_HARM0 + 8] = harm
    consts_h[0, C_HARM1:C_HARM1 + 8] = harm
    consts_h[0, C_IOTA3:C_IOTA3 + 3] = [0, 1, 2]
    consts_h[0, C_CS:C_CS + 3] = temp * ins["comp_scale"]
    consts_h[0, C_CB:C_CB + 3] = ins["comp_bias"]
    consts_h[0, C_TPW:C_TPW + 32] = ins["time_proj_w"][0]
    consts_h[0, C_TPB:C_TPB + 32] = ins["time_proj_b"]
    consts_h[0, C_EMB0:C_EMB0 + 8] = ins["comp_emb"][0]
    consts_h[0, C_EMB1:C_EMB1 + 8] = ins["comp_emb"][1]
    consts_h[0, C_EMB2:C_EMB2 + 8] = ins["comp_emb"][2]
    consts_h[0, C_OFFS:C_OFFS + 32] = np.tile(
        np.concatenate([np.zeros(8, np.float32), np.full(8, 0.25, np.float32)]), 2)
    iota12_h = np.arange(12, dtype=np.float32).reshape(12, 1)
    expander_h = np.full((12, 768), NEG, np.float32)
    for s in range(12):
        expander_h[s, s * 64:(s + 1) * 64] = 0.0
    expander_h = expander_h.astype(BF16)

    shared = dict(
        wk=wk_h, wv=wv_h, trunkw=trunkw_h, bqw=bq_h, cw1w=cw1_h, cw2w=cw2_h,
        tow=tow_h, bpw=bpw_h, rowb=rowb_h, expander=expander_h, ppb=ppb_h,
        cvrow=cvrow_h, consts=consts_h, iota12=iota12_h,
        ident=np.eye(128, dtype=BF16), ones=np.ones((1, 128), BF16),
        onesf=np.ones((1, 128), np.float32),
    )

    in_maps = []
    slotmaps = []
    for lo, tiles in cores:
        hb = np.zeros((B, K, D), np.float32)
        nb = min(B, T - lo)
        hb[:nb] = h[lo:lo + nb]
        ht_h = np.ascontiguousarray(
            hb.reshape(B64, D).T.reshape(2, 128, B64).transpose(1, 0, 2).reshape(128, 2 * B64)
        ).astype(BF16)
        stw_h = np.full((1, B), 1e9, np.float32)
        stw_h[0, :nb] = st[lo:lo + nb]
        qmeta_h = np.zeros((TPC, 128, 4), np.float32)
        grow_h = np.zeros((TPC, 128), np.float32)
        moff_h = np.zeros((1, TPC * 2), np.int32)
        smap = np.full((TPC, 128), -1, np.int64)
        for mth, (s, qsel, g, nreal) in enumerate(tiles):
            qmeta_h[mth, :, 0] = t_q[qsel]
            qmeta_h[mth, :, 1] = c[qsel]
            qmeta_h[mth, :, 2] = xy[qsel, 0]
            qmeta_h[mth, :, 3] = xy[qsel, 1]
            grow_h[mth] = g.astype(np.float32)
            moff_h[0, 2 * mth:2 * mth + 2] = [256 * s, 256 * s + 1024]
            smap[mth, :nreal] = qsel[:nreal]
        in_maps.append(dict(ht=ht_h, stw=stw_h, qmeta=qmeta_h, grow=grow_h,
                            moff=moff_h, **shared))
        slotmaps.append(smap.reshape(-1))
    return in_maps, slotmaps, B, TPC


_last_run = None


def kernel(**inputs):
    global _last_run
    in_maps, slotmaps, B, TPC = _prepare(inputs)
    nc = _build(B, TPC)
    _last_run = run_bass_kernel_spmd(nc, in_maps, list(range(NCORES)))
    results = _last_run.results

    out_full = np.zeros(N, np.float32)
    for ci in range(NCORES):
        o = np.asarray(results[ci]["out"]).T.reshape(-1)
        sm = slotmaps[ci]
        valid = sm >= 0
        out_full[sm[valid]] = o[valid]
    return out_full
